# revision 12
# baseline (speedup 1.0000x reference)
"""Trainium2 Bass kernel for BertFFN-with-memory-attention (scatter_memory).

Computation (see reference):
  h = LN(x, g1, b1); d = h @ W_down + b_down; q = LN(d, g2, b2)
  key = memory @ W_k + b_k; val = memory @ W_v + b_v
  s = softmax(q @ key^T); u = s @ val; w = LN(u, g3, b3)
  out = w @ W_up + b_up

Strategy: pure data parallel over batch (8 cores x 8 batches), with all the
tiny parameter algebra folded on the host into augmented matmul constants:
  d      = r1*(x@W1 - m1*c1) + e1          W1 = g1 (.) W_down (column-scaled)
  logits = q'@Kp - (r2*m2)*cK  (+eK)       q' = r2*d, Kp = g2 (.) key^T
  u_raw  = exp(logits) @ val'              val' = exp(eK) (.) val  (eK fold)
  out    = r3*(u_raw@W3 - m3*c3) + e3      softmax denom cancels in LN3
All matmuls run with bf16 operands and fp32 PSUM accumulation.
"""

import numpy as np
import ml_dtypes

import copy as _copy

import concourse.bass as bass
import concourse.tile as tile
from concourse import mybir
from concourse.alu_op_type import AluOpType
from concourse.bass_utils import run_bass_kernel_spmd

BF16 = mybir.dt.bfloat16
F32 = mybir.dt.float32
F32R = mybir.dt.float32r
AF = mybir.ActivationFunctionType

N_CORES = 8
B, S, H, D, M = 64, 512, 768, 16, 50
TOK = B * S                    # 32768 total tokens
TPC = TOK // N_CORES           # 4096 tokens per core
P = 128                        # partitions / tokens per tile
NTILE = TPC // P               # 32 tiles per core
TILES_PER_CHUNK = 4            # tiles per DMA (1.5 MB transfers)
NCHUNK = NTILE // TILES_PER_CHUNK
HC = H // P                    # 6 contraction chunks for the down-proj
EPS = 1e-12

_CACHE = {}


def _split_ctrl_waits(nc):
    """walrus for trn2 encodes at most ONE sync wait on CTRL-class
    instructions (Drain/branch/event-semaphore).  The Tile kernel-tail drain
    aggregates a wait per outstanding semaphore, which trips
    'Too many sync wait commands'.  Hoist excess waits onto injected
    same-engine Drain copies placed immediately before."""
    uid = [0]
    for f in nc.m.functions:
        for b in f.blocks:
            new_insts, changed = [], False
            for inst in b.instructions:
                si = inst.sync_info
                waits = list(si.on_wait) if si and si.on_wait else []
                tn = type(inst).__name__
                cap = _WAIT_CAP.get(tn, _WAIT_CAP["default"])
                if str(inst.engine) == "EngineType.Pool":
                    cap = 1
                if len(waits) > cap:
                    changed = True
                    for w in waits[:-cap]:
                        uid[0] += 1
                        new_insts.append(
                            _mk_drain(inst, f"{inst.name}_sw{uid[0]}", w)
                        )
                    inst.sync_info = mybir.SyncInfo(
                        on_wait=waits[-cap:],
                        on_update=list(si.on_update) if si.on_update else [],
                    )
                new_insts.append(inst)
            if changed:
                b.instructions = new_insts


_DRAIN_TEMPLATE = {}

_WAIT_CAP = {
    "InstEventSemaphore": 2,
    "InstDrain": 1,
    "InstUnconditionalBranch": 1,
    "InstISA": 1,
    "default": 1,
}


def _mk_drain(like_inst, name, wait):
    """Build a wait-carrier NOP on like_inst's engine with a single wait."""
    tmpl = _DRAIN_TEMPLATE.get(like_inst.engine)
    if tmpl is None:
        raise RuntimeError(f"no nop template for {like_inst.engine}")
    return _copy.replace(
        tmpl, name=name, sync_info=mybir.SyncInfo(on_wait=[wait], on_update=[])
    )


def _collect_drain_templates(nc):
    for f in nc.m.functions:
        for b in f.blocks:
            for inst in b.instructions:
                if type(inst).__name__ in ("InstNop", "InstISA") and (
                    "waitcarrier" in str(getattr(inst, "addl_debug", ""))
                    or "ENGINE_NOP" in str(getattr(inst, "opcode", ""))
                    or type(inst).__name__ == "InstNop"
                ):
                    _DRAIN_TEMPLATE.setdefault(inst.engine, inst)
            for inst in b.instructions:
                if type(inst).__name__ == "InstDrain":
                    _DRAIN_TEMPLATE.setdefault(inst.engine, inst)


def _build_program(has_e1, has_e3):
    nc = bass.Bass("TRN2", target_bir_lowering=False, debug=False)

    x_d = nc.dram_tensor("x", [TPC, H], F32, kind="ExternalInput")
    w1_d = nc.dram_tensor("w1", [H, D], BF16, kind="ExternalInput")
    kaug_d = nc.dram_tensor("kaug", [D + 1, M], BF16, kind="ExternalInput")
    val_d = nc.dram_tensor("valp", [M, D], F32R, kind="ExternalInput")
    ku = D + 1 + (1 if has_e3 else 0)
    w3_d = nc.dram_tensor("w3aug", [ku, H], F32R, kind="ExternalInput")
    ident_d = nc.dram_tensor("ident", [P, P], BF16, kind="ExternalInput")
    identf_d = nc.dram_tensor("identf", [P, P], F32, kind="ExternalInput")
    c1n_d = nc.dram_tensor("c1neg", [1, D], F32, kind="ExternalInput")
    e1_d = nc.dram_tensor("e1", [1, D], F32, kind="ExternalInput")
    y_d = nc.dram_tensor("y", [TPC, H], F32, kind="ExternalOutput")

    with tile.TileContext(nc) as tc:
        with (
            tc.tile_pool(name="consts", bufs=1) as consts,
            tc.tile_pool(name="xin", bufs=2) as xin,
            tc.tile_pool(name="yout", bufs=2) as yout,
            tc.tile_pool(name="xbf", bufs=2) as xbf,
            tc.tile_pool(name="xts", bufs=2) as xts,
            tc.tile_pool(name="ssb", bufs=3) as ssb,
            tc.tile_pool(name="stats", bufs=4) as stats,
            tc.tile_pool(name="xtp", bufs=1, space="PSUM") as xtp,
            tc.tile_pool(name="psm", bufs=3, space="PSUM") as psm,
            tc.tile_pool(name="pout", bufs=4, space="PSUM") as pout,
        ):
            w1_sb = consts.tile([P, HC, D], BF16)
            nc.sync.dma_start(
                out=w1_sb[:], in_=w1_d.ap().rearrange("(c p) d -> p c d", p=P)
            )
            kaug_sb = consts.tile([D + 1, M], BF16)
            nc.sync.dma_start(out=kaug_sb[:], in_=kaug_d.ap())
            val_sb = consts.tile([M, D], F32R)
            nc.sync.dma_start(out=val_sb[:], in_=val_d.ap())
            w3_sb = consts.tile([ku, H], F32R)
            nc.sync.dma_start(out=w3_sb[:], in_=w3_d.ap())
            ident = consts.tile([P, P], BF16)
            nc.sync.dma_start(out=ident[:], in_=ident_d.ap())
            identf = consts.tile([P, P], F32)
            nc.sync.dma_start(out=identf[:], in_=identf_d.ap())
            c1n_sb = consts.tile([P, D], F32)
            nc.sync.dma_start(out=c1n_sb[:], in_=c1n_d.ap().to_broadcast([P, D]))
            e1_sb = consts.tile([P, D], F32)
            if has_e1:
                nc.sync.dma_start(out=e1_sb[:], in_=e1_d.ap().to_broadcast([P, D]))
            eps_sb = consts.tile([P, 1], F32)
            nc.vector.memset(eps_sb[:], EPS)
            for _eng in (nc.vector, nc.scalar, nc.gpsimd, nc.tensor, nc.sync):
                _eng.nop(hint="waitcarrier", nofuse=True)

            for c in range(NCHUNK):
                rows = slice(c * TILES_PER_CHUNK * P, (c + 1) * TILES_PER_CHUNK * P)
                x_big = xin.tile([P, TILES_PER_CHUNK, H], F32)
                nc.sync.dma_start(
                    out=x_big[:],
                    in_=x_d.ap()[rows, :].rearrange("(t p) h -> p t h", p=P),
                )
                out_big = yout.tile([P, TILES_PER_CHUNK, H], F32)

                for t in range(TILES_PER_CHUNK):
                    xv = x_big[:, t, :]
                    x_bf = xbf.tile([P, H], BF16)
                    nc.scalar.copy(x_bf[:], xv)

                    # LN1 statistics (token-major, free-dim reduction)
                    st1 = stats.tile([P, 2, 6], BF16)
                    nc.vector.bn_stats(st1[:, 0, :], x_bf[:, 0 : H // 2])
                    nc.vector.bn_stats(st1[:, 1, :], x_bf[:, H // 2 : H])
                    mv1 = stats.tile([P, 2], F32)
                    nc.vector.bn_aggr(mv1[:], st1[:])
                    sd1 = stats.tile([P, 1], F32)
                    nc.scalar.activation(sd1[:], mv1[:, 1:2], AF.Sqrt, bias=eps_sb[:])
                    r1 = stats.tile([P, 1], F32)
                    nc.vector.reciprocal(r1[:], sd1[:])
                    a1 = stats.tile([P, 1], F32)
                    nc.vector.tensor_tensor(a1[:], mv1[:, 0:1], r1[:], AluOpType.mult)

                    # x^T via PE transposes -> PSUM -> SBUF
                    xT = xtp.tile([P, H], BF16)
                    for j in range(HC):
                        nc.tensor.transpose(
                            xT[:, j * P : (j + 1) * P],
                            x_bf[:, j * P : (j + 1) * P],
                            ident[:],
                        )
                    xTs = xts.tile([P, H], BF16)
                    nc.vector.tensor_copy(xTs[:], xT[:])

                    # praw^T = (x @ W1)^T accumulated over 6 K-chunks
                    prawT = psm.tile([D, P], F32, tag="sm")
                    for j in range(HC):
                        nc.tensor.matmul(
                            prawT[:],
                            w1_sb[:, j, :],
                            xTs[:, j * P : (j + 1) * P],
                            start=(j == 0),
                            stop=(j == HC - 1),
                        )
                    prawTs = ssb.tile([D, P], BF16)
                    nc.scalar.copy(prawTs[:], prawT[:])
                    praw = psm.tile([P, D], BF16, tag="sm")
                    nc.tensor.transpose(praw[:], prawTs[:], ident[0:D, 0:D])

                    # d = r1*praw - a1*c1 (+ e1)
                    s1 = ssb.tile([P, D], BF16)
                    nc.scalar.activation(s1[:], praw[:], AF.Copy, scale=r1[:])
                    d_sb = ssb.tile([P, D], BF16)
                    nc.vector.scalar_tensor_tensor(
                        d_sb[:], c1n_sb[:], a1[:], s1[:], AluOpType.mult, AluOpType.add
                    )
                    if has_e1:
                        nc.vector.tensor_tensor(
                            d_sb[:], d_sb[:], e1_sb[:], AluOpType.add
                        )

                    # LN2 stats -> q'aug = [r2*d, r2*m2]
                    st2 = stats.tile([P, 6], F32)
                    nc.vector.bn_stats(st2[:], d_sb[:])
                    mv2 = stats.tile([P, 2], F32)
                    nc.vector.bn_aggr(mv2[:], st2[:])
                    sd2 = stats.tile([P, 1], F32)
                    nc.scalar.activation(sd2[:], mv2[:, 1:2], AF.Sqrt, bias=eps_sb[:])
                    r2 = stats.tile([P, 1], F32)
                    nc.vector.reciprocal(r2[:], sd2[:])
                    qaug = ssb.tile([P, D + 1], BF16)
                    nc.scalar.activation(qaug[:, 0:D], d_sb[:], AF.Copy, scale=r2[:])
                    nc.vector.tensor_tensor(
                        qaug[:, D : D + 1], mv2[:, 0:1], r2[:], AluOpType.mult
                    )

                    qaugT = psm.tile([D + 1, P], BF16, tag="sm")
                    nc.tensor.transpose(qaugT[:], qaug[:], ident[:])
                    qaugTs = ssb.tile([D + 1, P], BF16)
                    nc.vector.tensor_copy(qaugTs[:], qaugT[:])

                    # logits^T = Kaug^T @ q'aug^T   [M, P]
                    scT = psm.tile([M, P], F32, tag="sm")
                    nc.tensor.matmul(
                        scT[:], kaug_sb[:], qaugTs[:], start=True, stop=True
                    )
                    pT = ssb.tile([M, P], F32R)
                    nc.scalar.activation(pT[:], scT[:], AF.Exp)

                    # u_raw = p~ @ val'   [P, D]
                    u_ps = psm.tile([P, D], F32, tag="sm")
                    nc.tensor.matmul(
                        u_ps[:],
                        pT[:],
                        val_sb[:],
                        start=True,
                        stop=True,
                    )

                    # LN3 stats -> uaug = [r3*u, r3*m3 (, 1)]
                    st3 = stats.tile([P, 6], F32)
                    nc.vector.bn_stats(st3[:], u_ps[:])
                    mv3 = stats.tile([P, 2], F32)
                    nc.vector.bn_aggr(mv3[:], st3[:])
                    sd3 = stats.tile([P, 1], F32)
                    nc.scalar.activation(sd3[:], mv3[:, 1:2], AF.Sqrt, bias=eps_sb[:])
                    r3 = stats.tile([P, 1], F32)
                    nc.vector.reciprocal(r3[:], sd3[:])
                    uaug = ssb.tile([P, ku], F32)
                    nc.scalar.activation(uaug[:, 0:D], u_ps[:], AF.Copy, scale=r3[:])
                    nc.vector.tensor_tensor(
                        uaug[:, D : D + 1], mv3[:, 0:1], r3[:], AluOpType.mult
                    )
                    if has_e3:
                        nc.gpsimd.memset(uaug[:, D + 1 : D + 2], 1.0)

                    uaugT = psm.tile([ku, P], F32, tag="sm")
                    nc.tensor.transpose(uaugT[:], uaug[:], identf[:])
                    uaugTs = ssb.tile([ku, P], F32R)
                    nc.vector.tensor_copy(uaugTs[:], uaugT[:])

                    # out = uaug @ W3aug   [P, H], split N=384+384
                    o1 = pout.tile([P, H // 2], F32, tag="po")
                    nc.tensor.matmul(
                        o1[:],
                        uaugTs[:],
                        w3_sb[:, 0 : H // 2],
                        start=True,
                        stop=True,
                    )
                    o2 = pout.tile([P, H // 2], F32, tag="po")
                    nc.tensor.matmul(
                        o2[:],
                        uaugTs[:],
                        w3_sb[:, H // 2 : H],
                        start=True,
                        stop=True,
                    )
                    nc.scalar.copy(out_big[:, t, 0 : H // 2], o1[:])
                    nc.vector.tensor_copy(out_big[:, t, H // 2 : H], o2[:])

                nc.sync.dma_start(
                    out=y_d.ap()[rows, :].rearrange("(t p) h -> p t h", p=P),
                    in_=out_big[:],
                )

    _collect_drain_templates(nc)
    _split_ctrl_waits(nc)
    return nc


def _fold_params(inputs):
    f = lambda k: np.asarray(inputs[k], np.float64)
    g1, b1 = f("g1"), f("b1")
    W_down, b_down = f("W_down"), f("b_down")
    g2, b2 = f("g2"), f("b2")
    memory, W_k, b_k = f("memory"), f("W_k"), f("b_k")
    W_v, b_v = f("W_v"), f("b_v")
    g3, b3 = f("g3"), f("b3")
    W_up, b_up = f("W_up"), f("b_up")

    key = memory @ W_k + b_k                     # [M, D]
    val = memory @ W_v + b_v                     # [M, D]
    W1 = g1[:, None] * W_down                    # [H, D]
    c1 = g1 @ W_down                             # [D]
    e1 = b1 @ W_down + b_down                    # [D]
    Kp = g2[:, None] * key.T                     # [D, M]
    cK = key @ g2                                # [M]
    eK = key @ b2                                # [M]
    W3 = g3[:, None] * W_up                      # [D, H]
    c3 = g3 @ W_up                               # [H]
    e3 = b3 @ W_up + b_up                        # [H]

    has_e1 = bool(np.max(np.abs(e1)) > 0)
    has_e3 = bool(np.max(np.abs(e3)) > 0)

    kaug = np.concatenate([Kp, -cK[None, :]], 0)              # [D+1, M]
    valp = np.exp(eK)[:, None] * val                          # [M, D]
    w3rows = [W3, -c3[None, :]] + ([e3[None, :]] if has_e3 else [])
    w3aug = np.concatenate(w3rows, 0)                         # [ku, H]

    bf = ml_dtypes.bfloat16
    return {
        "w1": W1.astype(bf),
        "kaug": kaug.astype(bf),
        "valp": valp.astype(np.float32),
        "w3aug": w3aug.astype(np.float32),
        "ident": np.eye(P, dtype=bf),
        "identf": np.eye(P, dtype=np.float32),
        "c1neg": (-c1[None, :]).astype(np.float32),
        "e1": e1[None, :].astype(np.float32),
    }, has_e1, has_e3


def kernel(**inputs):
    x = np.ascontiguousarray(
        np.asarray(inputs["hidden_states"], np.float32).reshape(TOK, H)
    )
    consts, has_e1, has_e3 = _fold_params(inputs)

    key = (has_e1, has_e3)
    if key not in _CACHE:
        _CACHE[key] = _build_program(has_e1, has_e3)
    nc = _CACHE[key]

    in_maps = [
        {"x": x[c * TPC : (c + 1) * TPC], **consts} for c in range(N_CORES)
    ]
    res = run_bass_kernel_spmd(nc, in_maps, list(range(N_CORES)))
    y = np.concatenate([res.results[c]["y"] for c in range(N_CORES)], axis=0)
    return y.reshape(B, S, H).astype(np.float32)


# revision 15
# speedup vs baseline: 1.2269x; 1.2269x over previous
"""Trainium2 Bass kernel for BertFFN-with-memory-attention (scatter_memory).

Computation (see reference):
  h = LN(x, g1, b1); d = h @ W_down + b_down; q = LN(d, g2, b2)
  key = memory @ W_k + b_k; val = memory @ W_v + b_v
  s = softmax(q @ key^T); u = s @ val; w = LN(u, g3, b3)
  out = w @ W_up + b_up

Pure data parallel over batch (8 cores x 4096 tokens).  Host folds all the
tiny parameter algebra into augmented matmul constants.  With e1 = b1@W_down
+ b_down == 0, LayerNorm scale-invariance removes the r1 chain entirely:
  z      = x@W1 - mean(x)*c1        (W1 = g1 (.) W_down, c1 = g1@W_down)
  logits = rz*(z@Kp - mz*cK) + eK   (rz = rsqrt(var(z)+eps); eK folded into
                                     val' = exp(eK) (.) val; softmax denom
                                     cancels inside LN3)
  u_raw  = exp(logits') @ val'
  out    = r3*(u_raw@W3 - m3*c3) + e3
rsqrt is computed as exp(-0.5*ln(v+eps)) so the scalar engine stays on one
activation-table set (ln/exp/copy).  Matmuls run bf16 on the down/attention
path and float32r on the output path, fp32 PSUM accumulation everywhere.
"""

import numpy as np
import ml_dtypes

import copy as _copy

import concourse.bass as bass
import concourse.tile as tile
from concourse import mybir
from concourse.alu_op_type import AluOpType
from concourse.bass_utils import run_bass_kernel_spmd

BF16 = mybir.dt.bfloat16
F32 = mybir.dt.float32
F32R = mybir.dt.float32r
AF = mybir.ActivationFunctionType

N_CORES = 8
B, S, H, D, M = 64, 512, 768, 16, 50
TOK = B * S                    # 32768 total tokens
TPC = TOK // N_CORES           # 4096 tokens per core
P = 128                        # partitions / tokens per tile
NTILE = TPC // P               # 32 tiles per core
TILES_PER_CHUNK = 4            # tiles per DMA (1.5 MB transfers)
NCHUNK = NTILE // TILES_PER_CHUNK
HC = H // P                    # 6 contraction chunks for the down-proj
EPS = 1e-12

_CACHE = {}
_NOP_TEMPLATE = {}


def _split_ctrl_waits(nc):
    """This walrus encodes at most ONE sync wait on most instruction
    templates (two on EventSemaphore).  Hoist excess waits onto injected
    same-engine NOPs placed immediately before the instruction."""
    uid = [0]
    for f in nc.m.functions:
        for b in f.blocks:
            new_insts, changed = [], False
            for inst in b.instructions:
                si = inst.sync_info
                waits = list(si.on_wait) if si and si.on_wait else []
                cap = 2 if type(inst).__name__ == "InstEventSemaphore" else 1
                if len(waits) > cap:
                    changed = True
                    tmpl = _NOP_TEMPLATE[inst.engine]
                    for w in waits[:-cap]:
                        uid[0] += 1
                        new_insts.append(
                            _copy.replace(
                                tmpl,
                                name=f"{inst.name}_sw{uid[0]}",
                                sync_info=mybir.SyncInfo(on_wait=[w], on_update=[]),
                            )
                        )
                    inst.sync_info = mybir.SyncInfo(
                        on_wait=waits[-cap:],
                        on_update=list(si.on_update) if si.on_update else [],
                    )
                new_insts.append(inst)
            if changed:
                b.instructions = new_insts


def _collect_nop_templates(nc):
    for f in nc.m.functions:
        for b in f.blocks:
            for inst in b.instructions:
                if type(inst).__name__ == "InstNoOp":
                    _NOP_TEMPLATE.setdefault(inst.engine, inst)
            for inst in b.instructions:  # fallback
                if type(inst).__name__ == "InstDrain":
                    _NOP_TEMPLATE.setdefault(inst.engine, inst)


def _build_program(has_e1, has_e3):
    nc = bass.Bass("TRN2", target_bir_lowering=False, debug=False)

    ku = D + 2 + (1 if has_e3 else 0)  # uaug cols: u, m3, v3 (, ones)

    x_d = nc.dram_tensor("x", [TPC, H], F32, kind="ExternalInput")
    w1_d = nc.dram_tensor("w1", [H, D], BF16, kind="ExternalInput")
    kaug_d = nc.dram_tensor("kaug", [D + 1, M], BF16, kind="ExternalInput")
    val_d = nc.dram_tensor("valp", [M, D], F32R, kind="ExternalInput")
    w3_d = nc.dram_tensor("w3aug", [ku, H], F32R, kind="ExternalInput")
    ident_d = nc.dram_tensor("ident", [P, P], BF16, kind="ExternalInput")
    identr_d = nc.dram_tensor("identr", [P, P], F32, kind="ExternalInput")
    c1n_d = nc.dram_tensor("c1negh", [1, D], F32, kind="ExternalInput")
    e1_d = nc.dram_tensor("e1", [1, D], F32, kind="ExternalInput")
    e3_d = nc.dram_tensor("e3", [1, H], F32, kind="ExternalInput")
    y_d = nc.dram_tensor("y", [TPC, H], F32, kind="ExternalOutput")

    with tile.TileContext(nc) as tc:
        with (
            tc.tile_pool(name="consts", bufs=1) as consts,
            tc.tile_pool(name="xin", bufs=2) as xin,
            tc.tile_pool(name="yout", bufs=2) as yout,
            tc.tile_pool(name="xbf", bufs=3) as xbf,
            tc.tile_pool(name="xts", bufs=3) as xts,
            tc.tile_pool(name="ssb", bufs=4) as ssb,
            tc.tile_pool(name="stats", bufs=6) as stats,
            tc.tile_pool(name="xtp", bufs=1, space="PSUM") as xtp,
            tc.tile_pool(name="psm", bufs=4, space="PSUM") as psm,
            tc.tile_pool(name="pout", bufs=3, space="PSUM") as pout,
        ):
            w1_sb = consts.tile([P, HC, D], BF16)
            nc.sync.dma_start(
                out=w1_sb[:], in_=w1_d.ap().rearrange("(c p) d -> p c d", p=P)
            )
            kaug_sb = consts.tile([D + 1, M], BF16)
            nc.sync.dma_start(out=kaug_sb[:], in_=kaug_d.ap())
            val_sb = consts.tile([M, D], F32R)
            nc.sync.dma_start(out=val_sb[:], in_=val_d.ap())
            w3_sb = consts.tile([ku, H], F32R)
            nc.sync.dma_start(out=w3_sb[:], in_=w3_d.ap())
            ident = consts.tile([P, P], BF16)
            nc.sync.dma_start(out=ident[:], in_=ident_d.ap())
            identr = consts.tile([P, P], F32)
            nc.sync.dma_start(out=identr[:], in_=identr_d.ap())
            c1n_sb = consts.tile([P, D], F32)
            nc.sync.dma_start(out=c1n_sb[:], in_=c1n_d.ap().to_broadcast([P, D]))
            e1_sb = consts.tile([P, D], F32)
            if has_e1:
                nc.sync.dma_start(out=e1_sb[:], in_=e1_d.ap().to_broadcast([P, D]))
            e3_sb = consts.tile([P, H], F32)
            if has_e3:
                nc.sync.dma_start(out=e3_sb[:], in_=e3_d.ap().to_broadcast([P, H]))
            eps_sb = consts.tile([P, 1], F32)
            nc.vector.memset(eps_sb[:], EPS)
            for _eng in (nc.vector, nc.scalar, nc.gpsimd, nc.tensor, nc.sync):
                _eng.nop(hint="waitcarrier", nofuse=True)

            for c in range(NCHUNK):
                rows = slice(c * TILES_PER_CHUNK * P, (c + 1) * TILES_PER_CHUNK * P)
                x_big = xin.tile([P, TILES_PER_CHUNK, H], F32)
                nc.sync.dma_start(
                    out=x_big[:],
                    in_=x_d.ap()[rows, :].rearrange("(t p) h -> p t h", p=P),
                )
                out_big = yout.tile([P, TILES_PER_CHUNK, H], F32)

                for t in range(TILES_PER_CHUNK):
                    xv = x_big[:, t, :]
                    # cast fp32 -> bf16, accumulating sum(x) on the side
                    x_bf = xbf.tile([P, H], BF16)
                    xsum = stats.tile([P, 1], F32)
                    nc.vector.tensor_scalar(
                        x_bf[:], xv, 1.0, 0.0, AluOpType.mult, AluOpType.add,
                        accum_out=xsum[:],
                    )

                    # x^T via PE transposes -> PSUM -> SBUF
                    xT = xtp.tile([P, H], BF16)
                    for j in range(HC):
                        nc.tensor.transpose(
                            xT[:, j * P : (j + 1) * P],
                            x_bf[:, j * P : (j + 1) * P],
                            ident[:],
                        )
                    xTs = xts.tile([P, H], BF16)
                    nc.vector.tensor_copy(xTs[:], xT[:])

                    # praw^T = (x @ W1)^T  [D, P], W1 chunks stationary
                    prawT = psm.tile([D, P], F32, tag="sm")
                    for j in range(HC):
                        nc.tensor.matmul(
                            prawT[:],
                            w1_sb[:, j, :],
                            xTs[:, j * P : (j + 1) * P],
                            start=(j == 0),
                            stop=(j == HC - 1),
                        )
                    prawTs = ssb.tile([D, P], BF16)
                    nc.scalar.copy(prawTs[:], prawT[:])
                    praw = psm.tile([P, D], BF16, tag="sm")
                    nc.tensor.transpose(praw[:], prawTs[:], ident[0:D, 0:D])

                    # z = praw - mean(x)*c1 (+ r1/e1 fallback when e1 != 0)
                    z = ssb.tile([P, D], BF16)
                    nc.vector.scalar_tensor_tensor(
                        z[:], c1n_sb[:], xsum[:], praw[:],
                        AluOpType.mult, AluOpType.add,
                    )
                    if has_e1:
                        # d = r1*z + e1 with r1 = rsqrt(var(x)+eps)
                        st1 = stats.tile([P, 2, 6], BF16)
                        nc.vector.bn_stats(st1[:, 0, :], x_bf[:, 0 : H // 2])
                        nc.vector.bn_stats(st1[:, 1, :], x_bf[:, H // 2 : H])
                        mv1 = stats.tile([P, 2], F32)
                        nc.vector.bn_aggr(mv1[:], st1[:])
                        l1 = stats.tile([P, 1], F32)
                        nc.scalar.activation(l1[:], mv1[:, 1:2], AF.Ln, bias=eps_sb[:])
                        r1 = stats.tile([P, 1], F32)
                        nc.scalar.activation(r1[:], l1[:], AF.Exp, scale=-0.5)
                        nc.vector.tensor_scalar(
                            z[:], z[:], r1[:], None, AluOpType.mult
                        )
                        nc.vector.tensor_tensor(z[:], z[:], e1_sb[:], AluOpType.add)

                    # LN2 stats of z -> rz
                    st2 = stats.tile([P, 6], BF16)
                    nc.vector.bn_stats(st2[:], z[:])
                    mv2 = stats.tile([P, 2], F32)
                    nc.vector.bn_aggr(mv2[:], st2[:])
                    l2 = stats.tile([P, 1], F32)
                    nc.scalar.activation(l2[:], mv2[:, 1:2], AF.Ln, bias=eps_sb[:])
                    rz = stats.tile([P, 1], F32)
                    nc.scalar.activation(rz[:], l2[:], AF.Exp, scale=-0.5)

                    # q'aug = [rz*z, rz*mz]
                    qaug = ssb.tile([P, D + 1], BF16)
                    nc.scalar.activation(qaug[:, 0:D], z[:], AF.Copy, scale=rz[:])
                    nc.vector.tensor_tensor(
                        qaug[:, D : D + 1], mv2[:, 0:1], rz[:], AluOpType.mult
                    )
                    qaugT = psm.tile([D + 1, P], BF16, tag="sm")
                    nc.tensor.transpose(qaugT[:], qaug[:], ident[:])
                    qaugTs = ssb.tile([D + 1, P], BF16)
                    nc.vector.tensor_copy(qaugTs[:], qaugT[:])

                    # logits^T [M, P] = Kaug^T @ q'aug^T ; p~^T = exp
                    scT = psm.tile([M, P], F32, tag="sm")
                    nc.tensor.matmul(
                        scT[:], kaug_sb[:], qaugTs[:], start=True, stop=True
                    )
                    pT = ssb.tile([M, P], F32R)
                    nc.scalar.activation(pT[:], scT[:], AF.Exp)

                    # u_raw = p~ @ val'  [P, D]
                    u_ps = psm.tile([P, D], F32, tag="sm")
                    nc.tensor.matmul(u_ps[:], pT[:], val_sb[:], start=True, stop=True)

                    # uaug = [u, m3, v3 (, 1)]
                    uaug = ssb.tile([P, ku], F32)
                    nc.scalar.copy(uaug[:, 0:D], u_ps[:])
                    st3 = stats.tile([P, 6], F32)
                    nc.vector.bn_stats(st3[:], u_ps[:])
                    nc.vector.bn_aggr(uaug[:, D : D + 2], st3[:])
                    if has_e3:
                        nc.gpsimd.memset(uaug[:, D + 2 : D + 3], 1.0)
                    l3 = stats.tile([P, 1], F32)
                    nc.scalar.activation(
                        l3[:], uaug[:, D + 1 : D + 2], AF.Ln, bias=eps_sb[:]
                    )
                    r3 = stats.tile([P, 1], F32)
                    nc.scalar.activation(r3[:], l3[:], AF.Exp, scale=-0.5)

                    uaugT = psm.tile([ku, P], F32, tag="sm")
                    nc.tensor.transpose(uaugT[:], uaug[:], identr[:])
                    uaugTs = ssb.tile([ku, P], F32R)
                    nc.vector.tensor_copy(uaugTs[:], uaugT[:])

                    # out = r3 * (uaug @ W3aug)   [P, H], split N=384+384
                    o1 = pout.tile([P, H // 2], F32, tag="po")
                    nc.tensor.matmul(
                        o1[:], uaugTs[:], w3_sb[:, 0 : H // 2], start=True, stop=True
                    )
                    o2 = pout.tile([P, H // 2], F32, tag="po")
                    nc.tensor.matmul(
                        o2[:], uaugTs[:], w3_sb[:, H // 2 : H], start=True, stop=True
                    )
                    ob1 = out_big[:, t, 0 : H // 2]
                    ob2 = out_big[:, t, H // 2 : H]
                    nc.scalar.activation(ob1, o1[:], AF.Copy, scale=r3[:])
                    nc.vector.tensor_scalar(
                        ob2, o2[:], r3[:], None, AluOpType.mult
                    )
                    if has_e3:
                        nc.vector.tensor_tensor(
                            ob1, ob1, e3_sb[:, 0 : H // 2], AluOpType.add
                        )
                        nc.vector.tensor_tensor(
                            ob2, ob2, e3_sb[:, H // 2 : H], AluOpType.add
                        )

                nc.sync.dma_start(
                    out=y_d.ap()[rows, :].rearrange("(t p) h -> p t h", p=P),
                    in_=out_big[:],
                )

    _collect_nop_templates(nc)
    _split_ctrl_waits(nc)
    return nc


def _fold_params(inputs):
    f = lambda k: np.asarray(inputs[k], np.float64)
    g1, b1 = f("g1"), f("b1")
    W_down, b_down = f("W_down"), f("b_down")
    g2, b2 = f("g2"), f("b2")
    memory, W_k, b_k = f("memory"), f("W_k"), f("b_k")
    W_v, b_v = f("W_v"), f("b_v")
    g3, b3 = f("g3"), f("b3")
    W_up, b_up = f("W_up"), f("b_up")

    key = memory @ W_k + b_k                     # [M, D]
    val = memory @ W_v + b_v                     # [M, D]
    W1 = g1[:, None] * W_down                    # [H, D]
    c1 = g1 @ W_down                             # [D]
    e1 = b1 @ W_down + b_down                    # [D]
    Kp = g2[:, None] * key.T                     # [D, M]
    cK = key @ g2                                # [M]
    eK = key @ b2                                # [M]
    W3 = g3[:, None] * W_up                      # [D, H]
    c3 = g3 @ W_up                               # [H]
    e3 = b3 @ W_up + b_up                        # [H]

    has_e1 = bool(np.max(np.abs(e1)) > 0)
    has_e3 = bool(np.max(np.abs(e3)) > 0)

    kaug = np.concatenate([Kp, -cK[None, :]], 0)              # [D+1, M]
    valp = np.exp(eK)[:, None] * val                          # [M, D]
    w3rows = [W3, -c3[None, :], np.zeros((1, H))] + (
        [e3[None, :]] if has_e3 else []
    )
    w3aug = np.concatenate(w3rows, 0)                         # [ku, H]

    bf = ml_dtypes.bfloat16
    return {
        "w1": W1.astype(bf),
        "kaug": kaug.astype(bf),
        "valp": valp.astype(np.float32),
        "w3aug": w3aug.astype(np.float32),
        "ident": np.eye(P, dtype=bf),
        "identr": np.eye(P, dtype=np.float32),
        "c1negh": (-c1[None, :] / H).astype(np.float32),
        "e1": e1[None, :].astype(np.float32),
        "e3": e3[None, :].astype(np.float32),
    }, has_e1, has_e3


def kernel(**inputs):
    x = np.ascontiguousarray(
        np.asarray(inputs["hidden_states"], np.float32).reshape(TOK, H)
    )
    consts, has_e1, has_e3 = _fold_params(inputs)

    key = (has_e1, has_e3)
    if key not in _CACHE:
        _CACHE[key] = _build_program(has_e1, has_e3)
    nc = _CACHE[key]

    in_maps = [
        {"x": x[c * TPC : (c + 1) * TPC], **consts} for c in range(N_CORES)
    ]
    res = run_bass_kernel_spmd(nc, in_maps, list(range(N_CORES)))
    y = np.concatenate([res.results[c]["y"] for c in range(N_CORES)], axis=0)
    return y.reshape(B, S, H).astype(np.float32)
